# revision 1
# baseline (speedup 1.0000x reference)
"""Trainium2 Bass kernel for EnhancedMultiHeadAttention (B=2, S=2048, DM=1024, H=16).

Sharding: 8 cores = 2 batches x 4 query-row blocks of 512. Each core computes
K/V for its whole batch (staged via DRAM), attention + output projection +
gate + layernorm for its 512 query rows. No collectives.

Matmuls run in fp32r (full PE speed, ~1.5e-4 per-matmul error); the
attn @ v step runs in bf16 so the two heads of a pair can be col-packed
into one PSUM bank (fp32r cannot target PSUM partitions 64-127).
"""
import math
import os
import sys

import numpy as np

for _p in ("/opt/trn_rl_repo", "/opt/pypackages"):
    if _p not in sys.path:
        sys.path.append(_p)

import concourse.bass as bass
import concourse.mybir as mybir
import concourse.tile as tile
from concourse import bacc
from concourse.bass_utils import run_bass_kernel_spmd

F32R = mybir.dt.float32r
F32 = mybir.dt.float32
BF16 = mybir.dt.bfloat16
AF = mybir.ActivationFunctionType
ALU = mybir.AluOpType

B, S, DM, H = 2, 2048, 1024, 16
HD = DM // H                  # 64
SQ = 512                      # query rows per core
NP = 128                      # partitions
KC = DM // NP                 # 8 contraction chunks
NT = S // NP                  # 16 key/value tiles
NPAIR = H // 2                # 8 head pairs
NST = SQ // NP                # 4 row tiles in row-layout phases
N512 = 512
SCALE = 1.0 / math.sqrt(HD)
EPS = 1e-5

_CACHE = {}
_TRACE = [False]
_LAST_RESULT = [None]


def _bcast(ap_1d, p=NP):
    return bass.AP(tensor=ap_1d.tensor, offset=ap_1d.offset,
                   ap=[[0, p]] + list(ap_1d.ap))


def _build():
    nc = bacc.Bacc("TRN2", target_bir_lowering=False, debug=False)

    xT_d = nc.dram_tensor("xT", [DM, S], F32R, kind="ExternalInput").ap()
    xq_d = nc.dram_tensor("xq", [DM, SQ], F32R, kind="ExternalInput").ap()
    xr_d = nc.dram_tensor("xr", [SQ, DM], F32, kind="ExternalInput").ap()
    wkT_d = nc.dram_tensor("wkT", [DM, DM], F32R, kind="ExternalInput").ap()
    wvT_d = nc.dram_tensor("wvT", [DM, DM], F32R, kind="ExternalInput").ap()
    wqT_d = nc.dram_tensor("wqT", [DM, DM], F32R, kind="ExternalInput").ap()
    woT_d = nc.dram_tensor("woT", [DM, DM], F32R, kind="ExternalInput").ap()
    wgT_d = nc.dram_tensor("wgT", [DM, DM], F32R, kind="ExternalInput").ap()
    bq_d = nc.dram_tensor("bq", [DM], F32, kind="ExternalInput").ap()
    bk_d = nc.dram_tensor("bk", [DM], F32, kind="ExternalInput").ap()
    bv_d = nc.dram_tensor("bv", [DM], F32, kind="ExternalInput").ap()
    bo_d = nc.dram_tensor("bo", [DM], F32, kind="ExternalInput").ap()
    bg_d = nc.dram_tensor("bg", [DM], F32, kind="ExternalInput").ap()
    gam_d = nc.dram_tensor("gam", [DM], F32, kind="ExternalInput").ap()
    bet_d = nc.dram_tensor("bet", [DM], F32, kind="ExternalInput").ap()
    y_d = nc.dram_tensor("y", [SQ, DM], F32, kind="ExternalOutput").ap()

    with tile.TileContext(nc) as tc:
        with tc.tile_pool(name="pers", bufs=1) as pers, \
             tc.tile_pool(name="dstage", bufs=1, space="DRAM") as dstage, \
             tc.tile_pool(name="resid", bufs=1) as resid:
            bq_sb = pers.tile([NP, KC], F32)
            bk_sb = pers.tile([NP, KC], F32)
            bo_sb = pers.tile([NP, KC], F32)
            nc.sync.dma_start(out=bq_sb, in_=bq_d.rearrange("(c p) -> p c", p=NP))
            nc.sync.dma_start(out=bk_sb, in_=bk_d.rearrange("(c p) -> p c", p=NP))
            nc.sync.dma_start(out=bo_sb, in_=bo_d.rearrange("(c p) -> p c", p=NP))
            bv_bc = pers.tile([NP, DM], F32)
            nc.sync.dma_start(out=bv_bc, in_=_bcast(bv_d))
            eps_sb = pers.tile([NP, 1], F32)
            nc.vector.memset(eps_sb, EPS)

            kT_st = dstage.tile([DM, S], F32R)
            v_st = dstage.tile([S, DM], BF16)

            # resident across attention + out-proj phases
            ctxT_sb = resid.tile([NP, NPAIR, SQ], F32R)
            qT_sb = resid.tile([NP, KC, SQ], F32R)

            # ---------------- phase 1+2: K / V projections ----------------
            with tc.tile_pool(name="xres", bufs=1) as xres, \
                 tc.tile_pool(name="wpool", bufs=2) as wpool, \
                 tc.tile_pool(name="stg", bufs=3) as stg, \
                 tc.tile_pool(name="pp", bufs=4, space="PSUM") as pp:
                xT_sb = xres.tile([NP, KC, S], F32R)
                nc.sync.dma_start(out=xT_sb,
                                  in_=xT_d.rearrange("(c p) s -> p c s", p=NP))
                wk_sb = wpool.tile([NP, KC, DM], F32R, tag="w")
                nc.sync.dma_start(out=wk_sb,
                                  in_=wkT_d.rearrange("(c p) d -> p c d", p=NP))
                # kT[d, t] = sum_k Wk[d, k] x[t, k] + bk[d]
                for dt in range(KC):
                    for ts in range(S // N512):
                        ps_t = pp.tile([NP, N512], F32, tag="pj")
                        for kc in range(KC):
                            nc.tensor.matmul(
                                ps_t,
                                wk_sb[:, kc, dt * NP:(dt + 1) * NP],
                                xT_sb[:, kc, ts * N512:(ts + 1) * N512],
                                start=(kc == 0), stop=(kc == KC - 1))
                        kt_t = stg.tile([NP, N512], F32R, tag="stf")
                        nc.vector.tensor_scalar_add(kt_t, ps_t, bk_sb[:, dt:dt + 1])
                        nc.sync.dma_start(
                            out=kT_st[dt * NP:(dt + 1) * NP, ts * N512:(ts + 1) * N512],
                            in_=kt_t)

                wv_sb = wpool.tile([NP, KC, DM], F32R, tag="w")
                nc.sync.dma_start(out=wv_sb,
                                  in_=wvT_d.rearrange("(c p) d -> p c d", p=NP))
                # v[t, d] = sum_k x[t, k] Wv_scaled[d, k] + bv_scaled[d]
                for tt in range(NT):
                    for ns in range(DM // N512):
                        ps_t = pp.tile([NP, N512], F32, tag="pj")
                        for kc in range(KC):
                            nc.tensor.matmul(
                                ps_t,
                                xT_sb[:, kc, tt * NP:(tt + 1) * NP],
                                wv_sb[:, kc, ns * N512:(ns + 1) * N512],
                                start=(kc == 0), stop=(kc == KC - 1))
                        v_t = stg.tile([NP, N512], BF16, tag="stb")
                        nc.vector.tensor_add(v_t, ps_t,
                                             bv_bc[:, ns * N512:(ns + 1) * N512])
                        nc.sync.dma_start(
                            out=v_st[tt * NP:(tt + 1) * NP, ns * N512:(ns + 1) * N512],
                            in_=v_t)

            # ---------------- phase 3: Q projection (transposed) -----------
            with tc.tile_pool(name="xqp", bufs=1) as xqp, \
                 tc.tile_pool(name="wq", bufs=1) as wqp, \
                 tc.tile_pool(name="ppq", bufs=4, space="PSUM") as ppq:
                xq_sb = xqp.tile([NP, KC, SQ], F32R)
                nc.sync.dma_start(out=xq_sb,
                                  in_=xq_d.rearrange("(c p) s -> p c s", p=NP))
                wq_sb = wqp.tile([NP, KC, DM], F32R)
                nc.sync.dma_start(out=wq_sb,
                                  in_=wqT_d.rearrange("(c p) d -> p c d", p=NP))
                for dt in range(KC):
                    ps_t = ppq.tile([NP, SQ], F32, tag="pjq")
                    for kc in range(KC):
                        nc.tensor.matmul(
                            ps_t,
                            wq_sb[:, kc, dt * NP:(dt + 1) * NP],
                            xq_sb[:, kc, :],
                            start=(kc == 0), stop=(kc == KC - 1))
                    nc.vector.tensor_scalar_add(qT_sb[:, dt, :], ps_t,
                                                bq_sb[:, dt:dt + 1])

            # ---------------- phase 4: attention ---------------------------
            with tc.tile_pool(name="kvp", bufs=2) as kvp, \
                 tc.tile_pool(name="attp", bufs=3) as attp, \
                 tc.tile_pool(name="scop", bufs=2, space="PSUM") as scop, \
                 tc.tile_pool(name="cxp", bufs=2, space="PSUM") as cxp:
                for p in range(NPAIR):
                    kpair = kvp.tile([NP, S], F32R, tag="kp")
                    nc.sync.dma_start(out=kpair, in_=kT_st[p * NP:(p + 1) * NP, :])
                    vpair = kvp.tile([NP, NT, NP], BF16, tag="vp")
                    nc.sync.dma_start(
                        out=vpair,
                        in_=v_st.rearrange("(t p) d -> p t d", p=NP)[:, :, p * NP:(p + 1) * NP])
                    ctx_ps = cxp.tile([NP, SQ], F32, tag="cx")
                    for t in range(NT):
                        sco = scop.tile([NP, 2 * SQ], F32, tag="sc")
                        nc.tensor.matmul(sco[:, 0:SQ],
                                         kpair[0:64, t * NP:(t + 1) * NP],
                                         qT_sb[0:64, p, :],
                                         start=True, stop=True, tile_position=(0, 0))
                        nc.tensor.matmul(sco[:, SQ:2 * SQ],
                                         kpair[64:128, t * NP:(t + 1) * NP],
                                         qT_sb[64:128, p, :],
                                         start=True, stop=True, tile_position=(64, 0))
                        att_t = attp.tile([NP, 2 * SQ], BF16, tag="at")
                        nc.scalar.activation(out=att_t, in_=sco, func=AF.Gelu,
                                             scale=SCALE)
                        nc.tensor.matmul(ctx_ps[0:64, :], vpair[:, t, 0:64],
                                         att_t[:, 0:SQ],
                                         start=(t == 0), stop=(t == NT - 1),
                                         tile_position=(0, 0))
                        nc.tensor.matmul(ctx_ps[64:128, :], vpair[:, t, 64:128],
                                         att_t[:, SQ:2 * SQ],
                                         start=(t == 0), stop=(t == NT - 1),
                                         tile_position=(0, 64))
                    nc.vector.tensor_copy(ctxT_sb[:, p, :], ctx_ps)

            # ---------------- phases 5-7: out proj, gate, epilogue ---------
            with tc.tile_pool(name="w2", bufs=2) as w2, \
                 tc.tile_pool(name="big", bufs=1) as big, \
                 tc.tile_pool(name="pp2", bufs=4, space="PSUM") as pp2:
                wo_sb = w2.tile([NP, KC, DM], F32R, tag="w2t")
                nc.sync.dma_start(out=wo_sb,
                                  in_=woT_d.rearrange("(c p) d -> p c d", p=NP))
                bo_bc = big.tile([NP, DM], F32)
                bg_bc = big.tile([NP, DM], F32)
                gam_bc = big.tile([NP, DM], F32)
                bet_bc = big.tile([NP, DM], F32)
                nc.sync.dma_start(out=bo_bc, in_=_bcast(bo_d))
                nc.sync.dma_start(out=bg_bc, in_=_bcast(bg_d))
                nc.sync.dma_start(out=gam_bc, in_=_bcast(gam_d))
                nc.sync.dma_start(out=bet_bc, in_=_bcast(bet_d))
                xr_sb = big.tile([NP, NST, DM], F32)
                nc.sync.dma_start(out=xr_sb,
                                  in_=xr_d.rearrange("(n p) d -> p n d", p=NP))

                # outT[d2, s] (lhsT for the gate matmul)
                outT_sb = big.tile([NP, KC, SQ], F32R)
                for dt in range(KC):
                    ps_t = pp2.tile([NP, SQ], F32, tag="po")
                    for dc in range(KC):
                        nc.tensor.matmul(
                            ps_t,
                            wo_sb[:, dc, dt * NP:(dt + 1) * NP],
                            ctxT_sb[:, dc, :],
                            start=(dc == 0), stop=(dc == KC - 1))
                    nc.vector.tensor_scalar_add(outT_sb[:, dt, :], ps_t,
                                                bo_sb[:, dt:dt + 1])

                # out_row[s, d2] (residual / epilogue layout)
                orow_sb = big.tile([NP, NST, DM], F32)
                for st in range(NST):
                    for ns in range(DM // N512):
                        ps_t = pp2.tile([NP, N512], F32, tag="po")
                        for dc in range(KC):
                            nc.tensor.matmul(
                                ps_t,
                                ctxT_sb[:, dc, st * NP:(st + 1) * NP],
                                wo_sb[:, dc, ns * N512:(ns + 1) * N512],
                                start=(dc == 0), stop=(dc == KC - 1))
                        nc.vector.tensor_add(
                            orow_sb[:, st, ns * N512:(ns + 1) * N512],
                            ps_t, bo_bc[:, ns * N512:(ns + 1) * N512])

                # gate_row[s, d3] = sigmoid(out @ Wg.T + bg)
                wg_sb = w2.tile([NP, KC, DM], F32R, tag="w2t")
                nc.sync.dma_start(out=wg_sb,
                                  in_=wgT_d.rearrange("(c p) d -> p c d", p=NP))
                gate_sb = big.tile([NP, NST, DM], F32)
                for st in range(NST):
                    for ns in range(DM // N512):
                        ps_t = pp2.tile([NP, N512], F32, tag="po")
                        for dc in range(KC):
                            nc.tensor.matmul(
                                ps_t,
                                outT_sb[:, dc, st * NP:(st + 1) * NP],
                                wg_sb[:, dc, ns * N512:(ns + 1) * N512],
                                start=(dc == 0), stop=(dc == KC - 1))
                        nc.vector.tensor_add(
                            gate_sb[:, st, ns * N512:(ns + 1) * N512],
                            ps_t, bg_bc[:, ns * N512:(ns + 1) * N512])
                nc.scalar.activation(out=gate_sb, in_=gate_sb, func=AF.Sigmoid)

                # y_pre = gate*out + (1-gate)*x + x = gate*(out - x) + 2x
                t1_sb = big.tile([NP, NST, DM], F32)
                nc.vector.tensor_sub(t1_sb, orow_sb, xr_sb)
                nc.vector.tensor_mul(orow_sb, t1_sb, gate_sb)
                nc.vector.scalar_tensor_tensor(
                    out=t1_sb, in0=xr_sb, scalar=2.0, in1=orow_sb,
                    op0=ALU.mult, op1=ALU.add)

                # layernorm over DM per row; output into gate_sb (dead)
                stats = pers.tile([NP, 2, 6], F32)
                mv = pers.tile([NP, 2], F32)
                std = pers.tile([NP, 1], F32)
                rstd = pers.tile([NP, 1], F32)
                y_sb = gate_sb
                for st in range(NST):
                    yv = t1_sb[:, st, :].rearrange("p (g d) -> p g d", g=2)
                    for g in range(2):
                        nc.vector.bn_stats(out=stats[:, g, :], in_=yv[:, g, :])
                    nc.vector.bn_aggr(out=mv, in_=stats)
                    nc.scalar.activation(out=std, in_=mv[:, 1:2], func=AF.Sqrt,
                                         bias=eps_sb)
                    nc.vector.reciprocal(rstd, std)
                    nc.vector.tensor_scalar(
                        out=orow_sb[:, st, :], in0=t1_sb[:, st, :],
                        scalar1=mv[:, 0:1], scalar2=rstd,
                        op0=ALU.subtract, op1=ALU.mult)
                    nc.vector.tensor_mul(orow_sb[:, st, :], orow_sb[:, st, :],
                                         gam_bc)
                    nc.vector.tensor_add(y_sb[:, st, :], orow_sb[:, st, :],
                                         bet_bc)
                nc.sync.dma_start(out=y_d.rearrange("(n p) d -> p n d", p=NP),
                                  in_=y_sb)

    nc.compile()
    return nc


def kernel(x, Wq, bq, Wk, bk, Wv, bv, Wo, bo, Wg, bg, attention_weights,
           ln_gamma, ln_beta):
    x = np.asarray(x, dtype=np.float32)
    f32 = lambda a: np.ascontiguousarray(np.asarray(a, dtype=np.float32))
    Wq, Wk, Wv, Wo, Wg = map(f32, (Wq, Wk, Wv, Wo, Wg))
    bq, bk, bv, bo, bg = map(f32, (bq, bk, bv, bo, bg))
    aw, gam, bet = map(f32, (attention_weights, ln_gamma, ln_beta))

    if "nc" not in _CACHE:
        _CACHE["nc"] = _build()
    nc = _CACHE["nc"]

    # fold softmax(attention_weights) into Wv / bv
    e = np.exp(aw - aw.max())
    head_w = (e / e.sum()).astype(np.float32)
    hw_exp = np.repeat(head_w, HD)              # [DM]
    Wv_s = Wv * hw_exp[:, None]
    bv_s = bv * hw_exp

    wqT = np.ascontiguousarray(Wq.T)
    wkT = np.ascontiguousarray(Wk.T)
    wvT = np.ascontiguousarray(Wv_s.T)
    woT = np.ascontiguousarray(Wo.T)
    wgT = np.ascontiguousarray(Wg.T)

    in_maps = []
    for c in range(8):
        b, blk = divmod(c, 4)
        r0 = blk * SQ
        xb = x[b]
        in_maps.append({
            "xT": np.ascontiguousarray(xb.T),
            "xq": np.ascontiguousarray(xb[r0:r0 + SQ].T),
            "xr": np.ascontiguousarray(xb[r0:r0 + SQ]),
            "wkT": wkT, "wvT": wvT, "wqT": wqT, "woT": woT, "wgT": wgT,
            "bq": bq, "bk": bk, "bv": bv_s, "bo": bo, "bg": bg,
            "gam": gam, "bet": bet,
        })

    res = run_bass_kernel_spmd(nc, in_maps, core_ids=list(range(8)),
                               trace=_TRACE[0])
    _LAST_RESULT[0] = res

    y = np.empty((B, S, DM), dtype=np.float32)
    for c in range(8):
        b, blk = divmod(c, 4)
        r0 = blk * SQ
        y[b, r0:r0 + SQ] = res.results[c]["y"]
    return y


# revision 4
# speedup vs baseline: 1.2414x; 1.2414x over previous
"""Trainium2 Bass kernel for EnhancedMultiHeadAttention (B=2, S=2048, DM=1024, H=16).

Sharding: 8 cores = 2 batches x 4 query-row blocks of 512. Each core computes
K/V for its whole batch, attention + output projection + gate + layernorm for
its 512 query rows. No collectives.

v2: K/Q projections computed just-in-time per head pair and interleaved with
attention, so the PE's projection matmuls fill the gaps under the ScalarE
GELU (the serial bottleneck) and keep the PE clock un-throttled. V is
projected in two column halves (staged via DRAM in bf16); half 1 overlaps
attention on pairs 0-3.

Matmuls run in fp32r (full PE speed, ~1.5e-4 per-matmul error); the
attn @ v step runs in bf16 so the two heads of a pair can be col-packed
into one PSUM bank (fp32r cannot target PSUM partitions 64-127).
"""
import math
import os
import sys

import numpy as np

for _p in ("/opt/trn_rl_repo", "/opt/pypackages"):
    if _p not in sys.path:
        sys.path.append(_p)

import concourse.bass as bass
import concourse.mybir as mybir
import concourse.tile as tile
from concourse import bacc
from concourse.bass_utils import run_bass_kernel_spmd

F32R = mybir.dt.float32r
F32 = mybir.dt.float32
BF16 = mybir.dt.bfloat16
AF = mybir.ActivationFunctionType
ALU = mybir.AluOpType

B, S, DM, H = 2, 2048, 1024, 16
HD = DM // H                  # 64
SQ = 512                      # query rows per core
NP = 128                      # partitions
KC = DM // NP                 # 8 contraction chunks
NT = S // NP                  # 16 key/value tiles
NPAIR = H // 2                # 8 head pairs
NST = SQ // NP                # 4 row tiles in row-layout phases
N512 = 512
SCALE = 1.0 / math.sqrt(HD)
EPS = 1e-5

_CACHE = {}
_TRACE = [False]
_LAST_RESULT = [None]


def _bcast(ap_1d, p=NP):
    return bass.AP(tensor=ap_1d.tensor, offset=ap_1d.offset,
                   ap=[[0, p]] + list(ap_1d.ap))


def _build():
    nc = bacc.Bacc("TRN2", target_bir_lowering=False, debug=False)

    xT_d = nc.dram_tensor("xT", [DM, S], F32R, kind="ExternalInput").ap()
    xq_d = nc.dram_tensor("xq", [DM, SQ], F32R, kind="ExternalInput").ap()
    xr_d = nc.dram_tensor("xr", [SQ, DM], F32, kind="ExternalInput").ap()
    wkT_d = nc.dram_tensor("wkT", [DM, DM], F32R, kind="ExternalInput").ap()
    wvT_d = nc.dram_tensor("wvT", [DM, DM], F32R, kind="ExternalInput").ap()
    wqT_d = nc.dram_tensor("wqT", [DM, DM], F32R, kind="ExternalInput").ap()
    woT_d = nc.dram_tensor("woT", [DM, DM], F32R, kind="ExternalInput").ap()
    wgT_d = nc.dram_tensor("wgT", [DM, DM], F32R, kind="ExternalInput").ap()
    bq_d = nc.dram_tensor("bq", [DM], F32, kind="ExternalInput").ap()
    bk_d = nc.dram_tensor("bk", [DM], F32, kind="ExternalInput").ap()
    bv_d = nc.dram_tensor("bv", [DM], F32, kind="ExternalInput").ap()
    bo_d = nc.dram_tensor("bo", [DM], F32, kind="ExternalInput").ap()
    bg_d = nc.dram_tensor("bg", [DM], F32, kind="ExternalInput").ap()
    gam_d = nc.dram_tensor("gam", [DM], F32, kind="ExternalInput").ap()
    bet_d = nc.dram_tensor("bet", [DM], F32, kind="ExternalInput").ap()
    y_d = nc.dram_tensor("y", [SQ, DM], F32, kind="ExternalOutput").ap()

    # views with 128-partition chunking
    xT_v = xT_d.rearrange("(c p) s -> p c s", p=NP)
    xq_v = xq_d.rearrange("(c p) s -> p c s", p=NP)
    wk_v = wkT_d.rearrange("(c p) d -> p c d", p=NP)
    wv_v = wvT_d.rearrange("(c p) d -> p c d", p=NP)
    wq_v = wqT_d.rearrange("(c p) d -> p c d", p=NP)
    wo_v = woT_d.rearrange("(c p) d -> p c d", p=NP)
    wg_v = wgT_d.rearrange("(c p) d -> p c d", p=NP)

    with tile.TileContext(nc) as tc:
        with tc.tile_pool(name="pers", bufs=1) as pers, \
             tc.tile_pool(name="dstage", bufs=1, space="DRAM") as dstage, \
             tc.tile_pool(name="resid", bufs=1) as resid:
            bq_sb = pers.tile([NP, KC], F32)
            bk_sb = pers.tile([NP, KC], F32)
            bo_sb = pers.tile([NP, KC], F32)
            nc.sync.dma_start(out=bq_sb, in_=bq_d.rearrange("(c p) -> p c", p=NP))
            nc.sync.dma_start(out=bk_sb, in_=bk_d.rearrange("(c p) -> p c", p=NP))
            nc.sync.dma_start(out=bo_sb, in_=bo_d.rearrange("(c p) -> p c", p=NP))
            bv_bc = pers.tile([NP, DM], F32)
            nc.sync.dma_start(out=bv_bc, in_=_bcast(bv_d))
            eps_sb = pers.tile([NP, 1], F32)
            nc.vector.memset(eps_sb, EPS)

            # V staged via DRAM in two column halves (fine-grained deps)
            v_st = [dstage.tile([S, N512], BF16, tag=f"vst{h}", name=f"vst{h}")
                    for h in range(2)]

            ctxT_sb = resid.tile([NP, NPAIR, SQ], F32R)

            with tc.tile_pool(name="xres", bufs=1) as xres, \
                 tc.tile_pool(name="xqres", bufs=1) as xqres, \
                 tc.tile_pool(name="wvp", bufs=2) as wvp, \
                 tc.tile_pool(name="wsl", bufs=2) as wsl, \
                 tc.tile_pool(name="kqp", bufs=2) as kqp, \
                 tc.tile_pool(name="vpp", bufs=2) as vpp, \
                 tc.tile_pool(name="attp", bufs=3) as attp, \
                 tc.tile_pool(name="stg", bufs=3) as stg, \
                 tc.tile_pool(name="pp", bufs=2, space="PSUM") as pp, \
                 tc.tile_pool(name="scop", bufs=2, space="PSUM") as scop, \
                 tc.tile_pool(name="cxp", bufs=2, space="PSUM") as cxp:
                # xT loaded in per-chunk DMAs so compute can start early
                xT_sb = xres.tile([NP, KC, S], F32R)
                for kc in range(KC):
                    nc.sync.dma_start(out=xT_sb[:, kc, :], in_=xT_v[:, kc, :])
                xq_sb = xqres.tile([NP, KC, SQ], F32R)
                for kc in range(KC):
                    nc.sync.dma_start(out=xq_sb[:, kc, :], in_=xq_v[:, kc, :])

                def v_proj(half):
                    wv_sb = wvp.tile([NP, KC, N512], F32R, tag="wv")
                    nc.sync.dma_start(
                        out=wv_sb, in_=wv_v[:, :, half * N512:(half + 1) * N512])
                    for tt in range(NT):
                        ps_t = pp.tile([NP, N512], F32, tag="pj")
                        for kc in range(KC):
                            nc.tensor.matmul(
                                ps_t,
                                xT_sb[:, kc, tt * NP:(tt + 1) * NP],
                                wv_sb[:, kc, :],
                                start=(kc == 0), stop=(kc == KC - 1))
                        v_t = stg.tile([NP, N512], BF16, tag="stb")
                        nc.vector.tensor_add(
                            v_t, ps_t, bv_bc[:, half * N512:(half + 1) * N512])
                        nc.gpsimd.dma_start(
                            out=v_st[half][tt * NP:(tt + 1) * NP, :], in_=v_t)

                def pair_block(p):
                    # K rows for this pair, just in time:
                    # kpair[d, t] = sum_k Wk[d, k] x[t, k] + bk[d], d in pair rows
                    wk_sl = wsl.tile([NP, KC, NP], F32R, tag="wk")
                    nc.sync.dma_start(out=wk_sl,
                                      in_=wk_v[:, :, p * NP:(p + 1) * NP])
                    kpair = kqp.tile([NP, S], F32R, tag="kp")
                    for ts in range(S // N512):
                        ps_t = pp.tile([NP, N512], F32, tag="pj")
                        for kc in range(KC):
                            nc.tensor.matmul(
                                ps_t,
                                wk_sl[:, kc, :],
                                xT_sb[:, kc, ts * N512:(ts + 1) * N512],
                                start=(kc == 0), stop=(kc == KC - 1))
                        nc.vector.tensor_scalar_add(
                            kpair[:, ts * N512:(ts + 1) * N512], ps_t,
                            bk_sb[:, p:p + 1])
                    # Q rows for this pair
                    wq_sl = wsl.tile([NP, KC, NP], F32R, tag="wq")
                    nc.sync.dma_start(out=wq_sl,
                                      in_=wq_v[:, :, p * NP:(p + 1) * NP])
                    qpair = kqp.tile([NP, SQ], F32R, tag="qp")
                    ps_q = pp.tile([NP, SQ], F32, tag="pj")
                    for kc in range(KC):
                        nc.tensor.matmul(ps_q, wq_sl[:, kc, :], xq_sb[:, kc, :],
                                         start=(kc == 0), stop=(kc == KC - 1))
                    nc.vector.tensor_scalar_add(qpair, ps_q, bq_sb[:, p:p + 1])
                    # V columns for this pair (from DRAM stage, bf16)
                    vpair = vpp.tile([NP, NT, NP], BF16, tag="vp")
                    nc.gpsimd.dma_start(
                        out=vpair,
                        in_=v_st[p // 4].rearrange("(t q) d -> q t d", q=NP)[
                            :, :, (p % 4) * NP:(p % 4) * NP + NP])
                    # attention for the two heads (row-packed scores,
                    # col-packed bf16 ctx)
                    ctx_ps = cxp.tile([NP, SQ], F32, tag="cx")
                    for t in range(NT):
                        sco = scop.tile([NP, 2 * SQ], F32, tag="sc")
                        nc.tensor.matmul(sco[:, 0:SQ],
                                         kpair[0:64, t * NP:(t + 1) * NP],
                                         qpair[0:64, :],
                                         start=True, stop=True,
                                         tile_position=(0, 0))
                        nc.tensor.matmul(sco[:, SQ:2 * SQ],
                                         kpair[64:128, t * NP:(t + 1) * NP],
                                         qpair[64:128, :],
                                         start=True, stop=True,
                                         tile_position=(64, 0))
                        att_t = attp.tile([NP, 2 * SQ], BF16, tag="at")
                        nc.scalar.activation(out=att_t, in_=sco, func=AF.Gelu,
                                             scale=SCALE)
                        nc.tensor.matmul(ctx_ps[0:64, :], vpair[:, t, 0:64],
                                         att_t[:, 0:SQ],
                                         start=(t == 0), stop=(t == NT - 1),
                                         tile_position=(0, 0))
                        nc.tensor.matmul(ctx_ps[64:128, :], vpair[:, t, 64:128],
                                         att_t[:, SQ:2 * SQ],
                                         start=(t == 0), stop=(t == NT - 1),
                                         tile_position=(0, 64))
                    nc.vector.tensor_copy(ctxT_sb[:, p, :], ctx_ps)

                v_proj(0)
                for p in range(4):
                    pair_block(p)
                v_proj(1)
                for p in range(4, 8):
                    pair_block(p)

            # ---------------- out proj, gate, epilogue ---------------------
            with tc.tile_pool(name="w2", bufs=2) as w2, \
                 tc.tile_pool(name="big", bufs=1) as big, \
                 tc.tile_pool(name="pp2", bufs=4, space="PSUM") as pp2:
                # wo loaded in column chunks so outT dt=0 starts immediately
                wo_sb = w2.tile([NP, KC, DM], F32R, tag="w2t")
                for dt in range(KC):
                    nc.sync.dma_start(out=wo_sb[:, :, dt * NP:(dt + 1) * NP],
                                      in_=wo_v[:, :, dt * NP:(dt + 1) * NP])
                bo_bc = big.tile([NP, DM], F32)
                bg_bc = big.tile([NP, DM], F32)
                gam_bc = big.tile([NP, DM], F32)
                bet_bc = big.tile([NP, DM], F32)
                nc.sync.dma_start(out=bo_bc, in_=_bcast(bo_d))
                nc.sync.dma_start(out=bg_bc, in_=_bcast(bg_d))
                nc.sync.dma_start(out=gam_bc, in_=_bcast(gam_d))
                nc.sync.dma_start(out=bet_bc, in_=_bcast(bet_d))
                xr_sb = big.tile([NP, NST, DM], F32)
                nc.sync.dma_start(out=xr_sb,
                                  in_=xr_d.rearrange("(n p) d -> p n d", p=NP))

                # outT[d2, s] (lhsT for the gate matmul)
                outT_sb = big.tile([NP, KC, SQ], F32R)
                for dt in range(KC):
                    ps_t = pp2.tile([NP, SQ], F32, tag="po")
                    for dc in range(KC):
                        nc.tensor.matmul(
                            ps_t,
                            wo_sb[:, dc, dt * NP:(dt + 1) * NP],
                            ctxT_sb[:, dc, :],
                            start=(dc == 0), stop=(dc == KC - 1))
                    nc.vector.tensor_scalar_add(outT_sb[:, dt, :], ps_t,
                                                bo_sb[:, dt:dt + 1])

                # out_row[s, d2] (residual / epilogue layout)
                orow_sb = big.tile([NP, NST, DM], F32)
                for st in range(NST):
                    for ns in range(DM // N512):
                        ps_t = pp2.tile([NP, N512], F32, tag="po")
                        for dc in range(KC):
                            nc.tensor.matmul(
                                ps_t,
                                ctxT_sb[:, dc, st * NP:(st + 1) * NP],
                                wo_sb[:, dc, ns * N512:(ns + 1) * N512],
                                start=(dc == 0), stop=(dc == KC - 1))
                        nc.vector.tensor_add(
                            orow_sb[:, st, ns * N512:(ns + 1) * N512],
                            ps_t, bo_bc[:, ns * N512:(ns + 1) * N512])

                # gate + epilogue per row-tile
                wg_sb = w2.tile([NP, KC, DM], F32R, tag="w2t")
                for dt in range(KC):
                    nc.sync.dma_start(out=wg_sb[:, :, dt * NP:(dt + 1) * NP],
                                      in_=wg_v[:, :, dt * NP:(dt + 1) * NP])
                gate_sb = big.tile([NP, NST, DM], F32)
                t1_sb = big.tile([NP, NST, DM], F32)
                stats = pers.tile([NP, 2, 6], F32)
                mv = pers.tile([NP, 2], F32)
                std = pers.tile([NP, 1], F32)
                rstd = pers.tile([NP, 1], F32)
                y_sb = gate_sb
                for st in range(NST):
                    for ns in range(DM // N512):
                        ps_t = pp2.tile([NP, N512], F32, tag="po")
                        for dc in range(KC):
                            nc.tensor.matmul(
                                ps_t,
                                outT_sb[:, dc, st * NP:(st + 1) * NP],
                                wg_sb[:, dc, ns * N512:(ns + 1) * N512],
                                start=(dc == 0), stop=(dc == KC - 1))
                        nc.vector.tensor_add(
                            gate_sb[:, st, ns * N512:(ns + 1) * N512],
                            ps_t, bg_bc[:, ns * N512:(ns + 1) * N512])
                    nc.scalar.activation(out=gate_sb[:, st, :],
                                         in_=gate_sb[:, st, :], func=AF.Sigmoid)
                    # y_pre = gate*(out - x) + 2x
                    nc.vector.tensor_sub(t1_sb[:, st, :], orow_sb[:, st, :],
                                         xr_sb[:, st, :])
                    nc.vector.tensor_mul(orow_sb[:, st, :], t1_sb[:, st, :],
                                         gate_sb[:, st, :])
                    nc.vector.scalar_tensor_tensor(
                        out=t1_sb[:, st, :], in0=xr_sb[:, st, :], scalar=2.0,
                        in1=orow_sb[:, st, :], op0=ALU.mult, op1=ALU.add)
                    # layernorm over DM
                    yv = t1_sb[:, st, :].rearrange("p (g d) -> p g d", g=2)
                    for g in range(2):
                        nc.vector.bn_stats(out=stats[:, g, :], in_=yv[:, g, :])
                    nc.vector.bn_aggr(out=mv, in_=stats)
                    nc.scalar.activation(out=std, in_=mv[:, 1:2], func=AF.Sqrt,
                                         bias=eps_sb)
                    nc.vector.reciprocal(rstd, std)
                    nc.vector.tensor_scalar(
                        out=orow_sb[:, st, :], in0=t1_sb[:, st, :],
                        scalar1=mv[:, 0:1], scalar2=rstd,
                        op0=ALU.subtract, op1=ALU.mult)
                    nc.vector.tensor_mul(orow_sb[:, st, :], orow_sb[:, st, :],
                                         gam_bc)
                    nc.vector.tensor_add(y_sb[:, st, :], orow_sb[:, st, :],
                                         bet_bc)
                    nc.sync.dma_start(
                        out=y_d.rearrange("(n p) d -> p n d", p=NP)[:, st, :],
                        in_=y_sb[:, st, :])

    nc.compile()
    return nc


def kernel(x, Wq, bq, Wk, bk, Wv, bv, Wo, bo, Wg, bg, attention_weights,
           ln_gamma, ln_beta):
    x = np.asarray(x, dtype=np.float32)
    f32 = lambda a: np.ascontiguousarray(np.asarray(a, dtype=np.float32))
    Wq, Wk, Wv, Wo, Wg = map(f32, (Wq, Wk, Wv, Wo, Wg))
    bq, bk, bv, bo, bg = map(f32, (bq, bk, bv, bo, bg))
    aw, gam, bet = map(f32, (attention_weights, ln_gamma, ln_beta))

    if "nc" not in _CACHE:
        _CACHE["nc"] = _build()
    nc = _CACHE["nc"]

    # fold softmax(attention_weights) into Wv / bv
    e = np.exp(aw - aw.max())
    head_w = (e / e.sum()).astype(np.float32)
    hw_exp = np.repeat(head_w, HD)              # [DM]
    Wv_s = Wv * hw_exp[:, None]
    bv_s = bv * hw_exp

    wqT = np.ascontiguousarray(Wq.T)
    wkT = np.ascontiguousarray(Wk.T)
    wvT = np.ascontiguousarray(Wv_s.T)
    woT = np.ascontiguousarray(Wo.T)
    wgT = np.ascontiguousarray(Wg.T)

    in_maps = []
    for c in range(8):
        b, blk = divmod(c, 4)
        r0 = blk * SQ
        xb = x[b]
        in_maps.append({
            "xT": np.ascontiguousarray(xb.T),
            "xq": np.ascontiguousarray(xb[r0:r0 + SQ].T),
            "xr": np.ascontiguousarray(xb[r0:r0 + SQ]),
            "wkT": wkT, "wvT": wvT, "wqT": wqT, "woT": woT, "wgT": wgT,
            "bq": bq, "bk": bk, "bv": bv_s, "bo": bo, "bg": bg,
            "gam": gam, "bet": bet,
        })

    res = run_bass_kernel_spmd(nc, in_maps, core_ids=list(range(8)),
                               trace=_TRACE[0])
    _LAST_RESULT[0] = res

    y = np.empty((B, S, DM), dtype=np.float32)
    for c in range(8):
        b, blk = divmod(c, 4)
        r0 = blk * SQ
        y[b, r0:r0 + SQ] = res.results[c]["y"]
    return y
